# revision 1
# baseline (speedup 1.0000x reference)
"""Decoder layer on 8 trn2 cores.

Sharding: data-parallel over (batch b, sequence half h) -> core c = 2*b + h.
Each core computes the full decoder layer for its 1024 tokens; K/V are
recomputed for the visible prefix (zero collectives). The per-core KV buffer
is [own 1024 tokens | first-half 1024 tokens]; the prefix block is gated
on/off per core by an exp-bias input (0 or -1e30), so one uniform SPMD
program serves both halves. Causal masking inside the own block is
compile-time (PSUM-preloaded -inf triangles via identity matmul).

Everything runs transposed (xT [D, tokens]) so no on-chip transposes are
needed: LN stats via ones-column matmuls, QK^T consumes qT/kT directly, the
AV matmul emits attn-out pre-transposed, the softmax denominator comes from
a ones column appended to V, and residual adds ride along in PSUM via
identity-matmul preloads. Output returns [D, tok]; host transposes.
"""

import numpy as np

D = 1024
H = 16
DH = 64
TQ = 1024
TKV = 2048
DFF = 4096
EPS = 1e-5
NEG = -1.0e30
KT = D // 128

_CACHE = {}


def _build():
    if "nc" in _CACHE:
        return _CACHE["nc"]
    import concourse.mybir as mybir
    import concourse.tile as tile
    from concourse import bacc

    f32 = mybir.dt.float32
    f32r = mybir.dt.float32r
    Act = mybir.ActivationFunctionType
    Alu = mybir.AluOpType

    nc = bacc.Bacc(None, target_bir_lowering=False)

    xkv = nc.declare_dram_parameter("xkv", [D, TKV], f32r, isOutput=False)
    pbias = nc.declare_dram_parameter("pbias", [128, 1], f32, isOutput=False)
    masks = nc.declare_dram_parameter("masks", [4, 128, 512], f32r, isOutput=False)
    ident = nc.declare_dram_parameter("ident", [128, 128], f32r, isOutput=False)
    onescol = nc.declare_dram_parameter("onescol", [128, 1], f32r, isOutput=False)
    onesrow = nc.declare_dram_parameter("onesrow", [1, 128], f32r, isOutput=False)
    wq = nc.declare_dram_parameter("wq", [D, D], f32r, isOutput=False)
    wk = nc.declare_dram_parameter("wk", [D, D], f32r, isOutput=False)
    wv = nc.declare_dram_parameter("wv", [D, D], f32r, isOutput=False)
    wo = nc.declare_dram_parameter("wo", [D, D], f32r, isOutput=False)
    wfc = nc.declare_dram_parameter("wfc", [D, DFF], f32r, isOutput=False)
    wpr = nc.declare_dram_parameter("wpr", [DFF, D], f32r, isOutput=False)
    bq = nc.declare_dram_parameter("bq", [1, D], f32r, isOutput=False)
    bk = nc.declare_dram_parameter("bk", [1, D], f32r, isOutput=False)
    bv = nc.declare_dram_parameter("bv", [1, D], f32r, isOutput=False)
    boT = nc.declare_dram_parameter("boT", [1, D], f32r, isOutput=False)
    bfcT = nc.declare_dram_parameter("bfcT", [1, DFF], f32r, isOutput=False)
    bprT = nc.declare_dram_parameter("bprT", [1, D], f32r, isOutput=False)
    ones512 = nc.declare_dram_parameter("ones512", [1, 512], f32r, isOutput=False)
    out_T = nc.declare_dram_parameter("out_T", [D, TQ], f32, isOutput=True)

    qT_s = nc.dram_tensor("qT_s", [D, TQ], f32r)
    kT_s = nc.dram_tensor("kT_s", [D, TKV], f32r)
    v_s = nc.dram_tensor("v_s", [TKV, D], f32r)
    x1_s = nc.dram_tensor("x1_s", [D, TQ], f32r)
    h2_s = nc.dram_tensor("h2_s", [D, TQ], f32r)

    with tile.TileContext(nc) as tc:
        with tc.tile_pool(name="const", bufs=1) as cst:
            id_t = cst.tile([128, 128], f32r)
            nc.sync.dma_start(out=id_t[:], in_=ident[:])
            ones_c = cst.tile([128, 1], f32r)
            nc.sync.dma_start(out=ones_c[:], in_=onescol[:])
            ones_r = cst.tile([1, 128], f32r)
            nc.sync.dma_start(out=ones_r[:], in_=onesrow[:])
            pb_t = cst.tile([128, 1], f32)
            nc.sync.dma_start(out=pb_t[:], in_=pbias[:])
            eps_row = cst.tile([1, 1024], f32)
            nc.any.memset(eps_row[:], EPS)
            ones5 = cst.tile([1, 512], f32r)
            nc.sync.dma_start(out=ones5[:], in_=ones512[:])
            mask_t = []
            for j in range(4):
                m = cst.tile([128, 512], f32r, tag=f"mask{j}")
                nc.sync.dma_start(out=m[:], in_=masks[j])
                mask_t.append(m)

            def ln_pools(s):
                return dict(
                    xk=s.enter_context(tc.tile_pool(name="xk", bufs=1)),
                    sq=s.enter_context(tc.tile_pool(name="sq", bufs=2)),
                    st=s.enter_context(tc.tile_pool(name="st", bufs=3)),
                    bc=s.enter_context(tc.tile_pool(name="bc", bufs=1)),
                    pst=s.enter_context(tc.tile_pool(name="pst", bufs=4, space="PSUM")),
                    pbc=s.enter_context(tc.tile_pool(name="pbc", bufs=2, space="PSUM")),
                )

            def layernorm(P, src, ncols, out_pool, tagp):
                """src: KT tiles [128, ncols] f32r -> KT tiles f32r normalized."""
                nch = ncols // 512
                ps1 = [P["pst"].tile([1, 512], f32, tag="st", name=f"ps1_{_i}") for _i in range(nch)]
                ps2 = [P["pst"].tile([1, 512], f32, tag="st", name=f"ps2_{_i}") for _i in range(nch)]
                for k in range(KT):
                    sqt = P["sq"].tile([128, ncols], f32r, tag="sqs")
                    nc.scalar.activation(sqt[:], src[k][:], Act.Square)
                    for c in range(nch):
                        nc.tensor.matmul(ps1[c][:], ones_c[:], src[k][:, c * 512:(c + 1) * 512],
                                         start=(k == 0), stop=(k == KT - 1))
                        nc.tensor.matmul(ps2[c][:], ones_c[:], sqt[:, c * 512:(c + 1) * 512],
                                         start=(k == 0), stop=(k == KT - 1))
                mu = P["st"].tile([1, ncols], f32r, tag="mu", bufs=1)
                ex2 = P["st"].tile([1, ncols], f32, tag="chain")
                for c in range(nch):
                    nc.scalar.mul(mu[:, c * 512:(c + 1) * 512], ps1[c][:], 1.0 / D)
                    nc.scalar.mul(ex2[:, c * 512:(c + 1) * 512], ps2[c][:], 1.0 / D)
                mu2 = P["st"].tile([1, ncols], f32, tag="chain")
                nc.vector.tensor_tensor(mu2[:], mu[:].bitcast(f32), mu[:].bitcast(f32), Alu.mult)
                var = P["st"].tile([1, ncols], f32, tag="chain")
                nc.vector.tensor_tensor(var[:], ex2[:], mu2[:], Alu.subtract)
                vre = P["st"].tile([1, ncols], f32, tag="chain")
                nc.vector.tensor_tensor(vre[:], var[:], eps_row[:, :ncols], Alu.add)
                vri = P["st"].tile([1, ncols], f32, tag="chain")
                nc.vector.reciprocal(vri[:], vre[:])
                rs = P["st"].tile([1, ncols], f32r, tag="rs", bufs=1)
                nc.scalar.activation(rs[:], vri[:], Act.Sqrt)
                mu_bc = P["bc"].tile([128, ncols], f32, tag="mubc")
                rs_bc = P["bc"].tile([128, ncols], f32, tag="rsbc")
                for c in range(nch):
                    pbb = P["pbc"].tile([128, 512], f32, tag="pbc")
                    nc.tensor.matmul(pbb[:], ones_r[:], mu[:, c * 512:(c + 1) * 512], start=True, stop=True)
                    nc.vector.tensor_copy(mu_bc[:, c * 512:(c + 1) * 512], pbb[:])
                    pbb2 = P["pbc"].tile([128, 512], f32, tag="pbc")
                    nc.tensor.matmul(pbb2[:], ones_r[:], rs[:, c * 512:(c + 1) * 512], start=True, stop=True)
                    nc.vector.tensor_copy(rs_bc[:, c * 512:(c + 1) * 512], pbb2[:])
                outs = []
                for k in range(KT):
                    t1 = P["sq"].tile([128, ncols], f32, tag="sqs")
                    nc.vector.tensor_tensor(t1[:], src[k][:].bitcast(f32), mu_bc[:], Alu.subtract)
                    ht = out_pool.tile([128, ncols], f32r, tag=f"{tagp}{k}")
                    nc.vector.tensor_tensor(ht[:], t1[:], rs_bc[:], Alu.mult)
                    outs.append(ht)
                return outs

            import contextlib

            # ============ Scope 1: LN1 + QKV projections (per token half)
            with contextlib.ExitStack() as s1:
                P = ln_pools(s1)
                htp = s1.enter_context(tc.tile_pool(name="ht", bufs=1))
                bqp = s1.enter_context(tc.tile_pool(name="bqp", bufs=1))
                bq_t = bqp.tile([1, D], f32r, tag="bqt")
                nc.sync.dma_start(out=bq_t[:], in_=bq[:])
                bk_t = bqp.tile([1, D], f32r, tag="bkt")
                nc.sync.dma_start(out=bk_t[:], in_=bk[:])
                bv_t = bqp.tile([1, D], f32r, tag="bvt")
                nc.sync.dma_start(out=bv_t[:], in_=bv[:])
                wvp = s1.enter_context(tc.tile_pool(name="wv", bufs=1))
                wsp = s1.enter_context(tc.tile_pool(name="ws", bufs=4))
                evp = s1.enter_context(tc.tile_pool(name="ev", bufs=4))
                pmm = s1.enter_context(tc.tile_pool(name="pmm", bufs=2, space="PSUM"))
                wv_res = []
                for k in range(KT):
                    wt = wvp.tile([128, D], f32r, tag=f"wv{k}")
                    nc.sync.dma_start(out=wt[:], in_=wv[k * 128:(k + 1) * 128, :])
                    wv_res.append(wt)
                for half in range(2):
                    xks = []
                    for k in range(KT):
                        xt = P["xk"].tile([128, TQ], f32r, tag=f"xk{k}")
                        nc.sync.dma_start(out=xt[:], in_=xkv[k * 128:(k + 1) * 128, half * TQ:(half + 1) * TQ])
                        xks.append(xt)
                    hts = layernorm(P, xks, TQ, htp, "ht")
                    if half == 0:
                        for mc in range(KT):
                            wts = []
                            for k in range(KT):
                                wt = wsp.tile([128, 128], f32r, tag="ws")
                                nc.sync.dma_start(out=wt[:], in_=wq[k * 128:(k + 1) * 128, mc * 128:(mc + 1) * 128])
                                wts.append(wt)
                            for c in range(2):
                                ps = pmm.tile([128, 512], f32, tag="pmm")
                                nc.tensor.matmul(ps[:], bq_t[:, mc * 128:(mc + 1) * 128], ones5[:],
                                                 start=True, stop=False)
                                for k in range(KT):
                                    nc.tensor.matmul(ps[:], wts[k][:], hts[k][:, c * 512:(c + 1) * 512],
                                                     start=False, stop=(k == KT - 1))
                                ev = evp.tile([128, 512], f32r, tag="ev")
                                nc.vector.tensor_copy(ev[:], ps[:])
                                nc.sync.dma_start(out=qT_s[mc * 128:(mc + 1) * 128, c * 512:(c + 1) * 512], in_=ev[:])
                    for cg in range(2):
                        c_glob = half * 2 + cg
                        for mc in range(KT):
                            ps = pmm.tile([128, 512], f32, tag="pmm")
                            nc.tensor.matmul(ps[:], bk_t[:, mc * 128:(mc + 1) * 128], ones5[:],
                                             start=True, stop=False)
                            for k in range(KT):
                                wt = wsp.tile([128, 128], f32r, tag="ws")
                                nc.sync.dma_start(out=wt[:], in_=wk[k * 128:(k + 1) * 128, mc * 128:(mc + 1) * 128])
                                nc.tensor.matmul(ps[:], wt[:], hts[k][:, cg * 512:(cg + 1) * 512],
                                                 start=False, stop=(k == KT - 1))
                            ev = evp.tile([128, 512], f32r, tag="ev")
                            nc.vector.tensor_copy(ev[:], ps[:])
                            nc.sync.dma_start(out=kT_s[mc * 128:(mc + 1) * 128, c_glob * 512:(c_glob + 1) * 512], in_=ev[:])
                    for tl in range(8):
                        tt = half * 8 + tl
                        for c in range(2):
                            ps = pmm.tile([128, 512], f32, tag="pmm")
                            nc.tensor.matmul(ps[:], ones_r[:], bv_t[:, c * 512:(c + 1) * 512],
                                             start=True, stop=False)
                            for k in range(KT):
                                nc.tensor.matmul(ps[:], hts[k][:, tl * 128:(tl + 1) * 128],
                                                 wv_res[k][:, c * 512:(c + 1) * 512],
                                                 start=False, stop=(k == KT - 1))
                            ev = evp.tile([128, 512], f32r, tag="ev")
                            nc.vector.tensor_copy(ev[:], ps[:])
                            nc.sync.dma_start(out=v_s[tt * 128:(tt + 1) * 128, c * 512:(c + 1) * 512], in_=ev[:])

            # ============ Scope 2: attention + output projection (-> x1_s)
            with contextlib.ExitStack() as s2:
                kqp = s2.enter_context(tc.tile_pool(name="kq", bufs=2))
                vap = s2.enter_context(tc.tile_pool(name="va", bufs=2))
                etp = s2.enter_context(tc.tile_pool(name="et", bufs=6))
                attp = s2.enter_context(tc.tile_pool(name="attn", bufs=1))
                stp2 = s2.enter_context(tc.tile_pool(name="st2", bufs=2))
                wop = s2.enter_context(tc.tile_pool(name="wo", bufs=4))
                xrp = s2.enter_context(tc.tile_pool(name="xr", bufs=2))
                evp2 = s2.enter_context(tc.tile_pool(name="ev2", bufs=4))
                pmm2 = s2.enter_context(tc.tile_pool(name="pm2", bufs=3, space="PSUM"))
                bo2p = s2.enter_context(tc.tile_pool(name="bo2", bufs=1))
                boT_t = bo2p.tile([1, D], f32r, tag="bot")
                nc.sync.dma_start(out=boT_t[:], in_=boT[:])
                pav = s2.enter_context(tc.tile_pool(name="pav", bufs=2, space="PSUM"))
                pbc2 = s2.enter_context(tc.tile_pool(name="pb2", bufs=2, space="PSUM"))
                attn_tiles = [attp.tile([128, TQ], f32r, tag=f"attn{i}", name=f"attnt{i}") for i in range(KT)]
                for h in range(H):
                    kTh = kqp.tile([64, TKV], f32r, tag="kTh")
                    nc.sync.dma_start(out=kTh[:], in_=kT_s[h * 64:(h + 1) * 64, :])
                    qTh = kqp.tile([64, TQ], f32r, tag="qTh")
                    nc.sync.dma_start(out=qTh[:], in_=qT_s[h * 64:(h + 1) * 64, :])
                    va = {}
                    for kt in range(16):
                        vt = vap.tile([128, 65], f32r, tag=f"va{kt}")
                        nc.sync.dma_start(out=vt[:, 0:64], in_=v_s[kt * 128:(kt + 1) * 128, h * 64:(h + 1) * 64])
                        nc.sync.dma_start(out=vt[:, 64:65], in_=onescol[:])
                        va[kt] = vt
                    for qc in range(2):
                        vis = list(range(4 * (qc + 1))) + list(range(8, 16))
                        ets = {}
                        for kt in vis:
                            ps_s = pmm2.tile([128, 512], f32, tag="ps_s")
                            bnd = 4 * qc <= kt < 4 * (qc + 1)
                            if bnd:
                                nc.tensor.matmul(ps_s[:], id_t[:], mask_t[kt - 4 * qc][:], start=True, stop=False)
                            nc.tensor.matmul(ps_s[:], kTh[:, kt * 128:(kt + 1) * 128],
                                             qTh[:, qc * 512:(qc + 1) * 512], start=(not bnd), stop=True)
                            et = etp.tile([128, 512], f32r, tag="et")
                            bias = pb_t[:, 0:1] if kt >= 8 else 0.0
                            nc.scalar.activation(et[:], ps_s[:], Act.Exp, bias=bias, scale=0.125)
                            ets[kt] = et
                        ps_av = pav.tile([65, 512], f32, tag="pav")
                        for i, kt in enumerate(vis):
                            nc.tensor.matmul(ps_av[:], va[kt][:], ets[kt][:],
                                             start=(i == 0), stop=(i == len(vis) - 1))
                        rec0 = stp2.tile([1, 512], f32, tag="rec0")
                        nc.vector.reciprocal(rec0[:], ps_av[64:65, :])
                        rec = stp2.tile([1, 512], f32r, tag="rec")
                        nc.scalar.activation(rec[:], rec0[:], Act.Copy)
                        ps_b = pbc2.tile([64, 512], f32, tag="pb64")
                        nc.tensor.matmul(ps_b[:], ones_r[:, 0:64], rec[:], start=True, stop=True)
                        bc_sb = stp2.tile([64, 512], f32, tag="bcsb")
                        nc.vector.tensor_copy(bc_sb[:], ps_b[:])
                        nc.vector.tensor_tensor(
                            attn_tiles[h // 2][(h % 2) * 64:(h % 2) * 64 + 64, qc * 512:(qc + 1) * 512],
                            ps_av[0:64, :], bc_sb[:], Alu.mult)
                # output projection + residual (residual preloaded into PSUM)
                for mc in range(KT):
                    xr = xrp.tile([128, TQ], f32r, tag="xr")
                    nc.sync.dma_start(out=xr[:], in_=xkv[mc * 128:(mc + 1) * 128, 0:TQ])
                    wts = []
                    for k in range(KT):
                        wt = wop.tile([128, 128], f32r, tag="wo")
                        nc.sync.dma_start(out=wt[:], in_=wo[k * 128:(k + 1) * 128, mc * 128:(mc + 1) * 128])
                        wts.append(wt)
                    for qc in range(2):
                        ps_o = pmm2.tile([128, 512], f32, tag="ps_s")
                        nc.tensor.matmul(ps_o[:], id_t[:], xr[:, qc * 512:(qc + 1) * 512], start=True, stop=False)
                        nc.tensor.matmul(ps_o[:], boT_t[:, mc * 128:(mc + 1) * 128], ones5[:],
                                         start=False, stop=False)
                        for k in range(KT):
                            nc.tensor.matmul(ps_o[:], wts[k][:], attn_tiles[k][:, qc * 512:(qc + 1) * 512],
                                             start=False, stop=(k == KT - 1))
                        ev = evp2.tile([128, 512], f32r, tag="ev")
                        nc.scalar.activation(ev[:], ps_o[:], Act.Copy)
                        nc.sync.dma_start(out=x1_s[mc * 128:(mc + 1) * 128, qc * 512:(qc + 1) * 512], in_=ev[:])

            # ============ Scope 3a: LN2 (x1_s -> h2_s)
            with contextlib.ExitStack() as s3:
                P = ln_pools(s3)
                h2p = s3.enter_context(tc.tile_pool(name="h2", bufs=1))
                evh = s3.enter_context(tc.tile_pool(name="evh", bufs=2))
                x1_tiles = []
                for k in range(KT):
                    xt = P["xk"].tile([128, TQ], f32r, tag=f"x1{k}")
                    nc.sync.dma_start(out=xt[:], in_=x1_s[k * 128:(k + 1) * 128, :])
                    x1_tiles.append(xt)
                h2_tiles = layernorm(P, x1_tiles, TQ, h2p, "h2")
                for k in range(KT):
                    nc.sync.dma_start(out=h2_s[k * 128:(k + 1) * 128, :], in_=h2_tiles[k][:])

            # ============ Scope 3b: MLP
            with contextlib.ExitStack() as s3:
                h2p = s3.enter_context(tc.tile_pool(name="h2b", bufs=1))
                x1p = s3.enter_context(tc.tile_pool(name="x1b", bufs=4))
                mtp = s3.enter_context(tc.tile_pool(name="mt", bufs=1))
                wmp = s3.enter_context(tc.tile_pool(name="wm", bufs=6))
                evp3 = s3.enter_context(tc.tile_pool(name="ev3", bufs=4))
                pmm3 = s3.enter_context(tc.tile_pool(name="pm3", bufs=2, space="PSUM"))
                bf3p = s3.enter_context(tc.tile_pool(name="bf3", bufs=1))
                bfcT_t = bf3p.tile([1, DFF], f32r, tag="bfct")
                nc.sync.dma_start(out=bfcT_t[:], in_=bfcT[:])
                bprT_t = bf3p.tile([1, D], f32r, tag="bprt")
                nc.sync.dma_start(out=bprT_t[:], in_=bprT[:])
                h2_tiles = []
                for k in range(KT):
                    xt = h2p.tile([128, TQ], f32r, tag=f"h2b{k}")
                    nc.sync.dma_start(out=xt[:], in_=h2_s[k * 128:(k + 1) * 128, :])
                    h2_tiles.append(xt)
                for qc in range(2):
                    mts = []
                    for hc in range(32):
                        ps_m = pmm3.tile([128, 512], f32, tag="pm3")
                        nc.tensor.matmul(ps_m[:], bfcT_t[:, hc * 128:(hc + 1) * 128], ones5[:],
                                         start=True, stop=False)
                        for k in range(KT):
                            wt = wmp.tile([128, 128], f32r, tag="wf")
                            nc.sync.dma_start(out=wt[:], in_=wfc[k * 128:(k + 1) * 128, hc * 128:(hc + 1) * 128])
                            nc.tensor.matmul(ps_m[:], wt[:], h2_tiles[k][:, qc * 512:(qc + 1) * 512],
                                             start=False, stop=(k == KT - 1))
                        mt = mtp.tile([128, 512], f32r, tag=f"mt{hc}")
                        nc.scalar.activation(mt[:], ps_m[:], Act.Gelu)
                        mts.append(mt)
                    for mc in range(KT):
                        x1r = x1p.tile([128, 512], f32r, tag="x1r")
                        nc.sync.dma_start(out=x1r[:], in_=x1_s[mc * 128:(mc + 1) * 128, qc * 512:(qc + 1) * 512])
                        ps_p = pmm3.tile([128, 512], f32, tag="pm3")
                        nc.tensor.matmul(ps_p[:], id_t[:], x1r[:], start=True, stop=False)
                        nc.tensor.matmul(ps_p[:], bprT_t[:, mc * 128:(mc + 1) * 128], ones5[:],
                                         start=False, stop=False)
                        for hc in range(32):
                            wt = wmp.tile([128, 128], f32r, tag="wp")
                            nc.sync.dma_start(out=wt[:], in_=wpr[hc * 128:(hc + 1) * 128, mc * 128:(mc + 1) * 128])
                            nc.tensor.matmul(ps_p[:], wt[:], mts[hc][:], start=False, stop=(hc == 31))
                        o = evp3.tile([128, 512], f32, tag="o")
                        nc.scalar.activation(o[:], ps_p[:], Act.Copy)
                        nc.sync.dma_start(out=out_T[mc * 128:(mc + 1) * 128, qc * 512:(qc + 1) * 512], in_=o[:])

    nc.compile()
    _CACHE["nc"] = nc
    return nc


def make_in_maps(x, ln1_g, ln1_b, wq, wk, wv, wo, bo, ln2_g, ln2_b, w_fc, b_fc, w_pr, b_pr):
    x = np.asarray(x, np.float32)
    mk = np.zeros((4, 128, 512), np.float32)
    for j in range(4):
        kp = np.arange(128)[:, None] + j * 128
        qf = np.arange(512)[None, :]
        mk[j] = np.where(kp <= qf, 0.0, NEG)
    shared = {
        "masks": mk,
        "ident": np.eye(128, dtype=np.float32),
        "onescol": np.ones((128, 1), np.float32),
        "onesrow": np.ones((1, 128), np.float32),
    }
    g1v = np.asarray(ln1_g, np.float32)
    b1v = np.asarray(ln1_b, np.float32)
    g2v = np.asarray(ln2_g, np.float32)
    b2v = np.asarray(ln2_b, np.float32)
    wq2 = np.transpose(np.asarray(wq, np.float32), (1, 0, 2)).reshape(D, D)
    wk2 = np.transpose(np.asarray(wk, np.float32), (1, 0, 2)).reshape(D, D)
    wv2 = np.transpose(np.asarray(wv, np.float32), (1, 0, 2)).reshape(D, D)
    wfc2 = np.asarray(w_fc, np.float32)
    shared["wq"] = np.ascontiguousarray(g1v[:, None] * wq2)
    shared["wk"] = np.ascontiguousarray(g1v[:, None] * wk2)
    shared["wv"] = np.ascontiguousarray(g1v[:, None] * wv2)
    shared["bq"] = (b1v @ wq2).reshape(1, D)
    shared["bk"] = (b1v @ wk2).reshape(1, D)
    shared["bv"] = (b1v @ wv2).reshape(1, D)
    shared["wo"] = np.ascontiguousarray(np.asarray(wo, np.float32))
    shared["boT"] = np.asarray(bo, np.float32).reshape(1, D)
    shared["wfc"] = np.ascontiguousarray(g2v[:, None] * wfc2)
    shared["bfcT"] = (np.asarray(b_fc, np.float32) + b2v @ wfc2).reshape(1, DFF)
    shared["wpr"] = np.ascontiguousarray(np.asarray(w_pr, np.float32))
    shared["bprT"] = np.asarray(b_pr, np.float32).reshape(1, D)
    shared["ones512"] = np.ones((1, 512), np.float32)
    in_maps = []
    for b in range(4):
        for h in range(2):
            own = x[b, h * TQ:(h + 1) * TQ]
            pref = x[b, 0:TQ]
            m = dict(shared)
            m["xkv"] = np.ascontiguousarray(np.concatenate([own, pref], 0).T)
            m["pbias"] = np.full((128, 1), 0.0 if h == 1 else NEG, np.float32)
            in_maps.append(m)
    return in_maps


def kernel(x, ln1_g, ln1_b, wq, wk, wv, wo, bo, ln2_g, ln2_b, w_fc, b_fc, w_pr, b_pr):
    from concourse.bass_utils import run_bass_kernel_spmd

    nc = _build()
    in_maps = make_in_maps(x, ln1_g, ln1_b, wq, wk, wv, wo, bo, ln2_g, ln2_b,
                           w_fc, b_fc, w_pr, b_pr)
    res = run_bass_kernel_spmd(nc, in_maps, list(range(8)))
    out = np.empty((4, 2048, D), np.float32)
    for b in range(4):
        for h in range(2):
            out[b, h * TQ:(h + 1) * TQ, :] = res.results[2 * b + h]["out_T"].T
    return out

